# revision 6
# baseline (speedup 1.0000x reference)
"""Trainium2 Bass kernel for nn_CustomDense: out = input @ weight.T.

Shapes (fp32): input [131072, 256], weight [256, 256], out [131072, 256].
Strategy: data-parallel over 8 NeuronCores — shard input rows (M) 8 ways,
replicate weight. Per core: out_loc[16384, 256] = a_loc @ w.T.

Per-core kernel:
  - one-time: load weight naturally ([n, k] rows on partitions), PE-transpose
    the four 128x128 sub-tiles into wt[k, n] layout in SBUF.
  - main loop over row chunks in a blocked layout (each SBUF partition holds
    `rows_per_part` consecutive DRAM rows, so DMA descriptors are
    rows_per_part KB of contiguous HBM per partition). Per stripe-pair
    (2 x 128 rows): PE-transpose the four k-tiles to [k, m] in one PSUM bank,
    round-cast to an f32r SBUF tile on DVE, accumulate the k-tile matmuls
    (lhsT=at[k,m], rhs=wt[k,n]) into PSUM, evict [m, n] to SBUF on ACT, and
    DMA the chunk out.

The kernel is HBM-DMA-bound (~33.6 MB/core over 16 SDMA engines at
~26 GB/s each), so the compute pipeline is software-pipelined to stay off
the critical path:
  - The PE program interleaves transpose(i+lag) before matmul(i), so the PE
    never idles waiting for the DVE eviction of at(i) — it transposes ahead.
  - PSUM evictions are split by role: at-tiles on DVE, mm outputs on ACT.
  - Stores ride the ACT HWDGE ring, emitted right after that chunk's last
    mm eviction on the same engine: the trigger's data dependency is already
    satisfied in program order, so it never blocks, and the SWDGE (gpsimd)
    descriptor-generation latency and ring contention are avoided entirely.
    Loads keep the SP HWDGE ring to stream both directions concurrently.

Matmuls run as float32r — 1 PE cycle/row at moving free dim >= 256 vs 4
cycles/row for plain fp32 (fp32 matmuls are 2 internal half-rate passes).
float32r rounds the operands (TF32-like), giving rel err ~1.2e-4 vs the
fp32 reference; mm_f32r=False selects exact fp32 at ~4x the PE cost.
"""

import numpy as np

import concourse.bass as bass
import concourse.mybir as mybir
import concourse.tile as tile
from concourse import bacc
from concourse.bass_utils import run_bass_kernel_spmd
from concourse.masks import make_identity

M, K, N = 131072, 256, 256
NCORES = 8
M_LOC = M // NCORES  # 16384 rows per core
P = 128
KT = K // P  # 2 k-tiles
NT = N // P  # 2 n-tiles

F32 = mybir.dt.float32
F32R = mybir.dt.float32r


def _chunk_schedule(r_total, rp):
    """r-slice sizes: small chunks at the ends to shorten pipeline fill/drain."""
    head = [2, 2, 4]
    tail = [4, 2, 2]
    mid = r_total - sum(head) - sum(tail)
    if mid < 0 or rp <= 4:
        assert r_total % rp == 0
        return [rp] * (r_total // rp)
    assert mid % rp == 0
    return head + [rp] * (mid // rp) + tail


def build_nc(
    m_loc=M_LOC,
    rows_per_part=8,
    lag=2,
    a_bufs=7,
    out_bufs=12,
    store_delay=8,
    store_scalar=True,
    mm_f32r=True,
    tr_f32r=True,
):
    """Build the per-core Bass program (SPMD: same program on all cores)."""
    rp = rows_per_part
    r_total = m_loc // P  # rows per partition over the whole kernel

    mm_dt = F32R if mm_f32r else F32
    # Rounding A to f32r during the transpose costs nothing extra in
    # precision (the cast to the f32r at-tile rounds anyway) and runs the
    # PE transpose at 1.5 cyc/row instead of 2.
    tr_dt = F32R if (mm_f32r and tr_f32r) else F32

    nc = bacc.Bacc("TRN2", target_bir_lowering=False, debug=False)

    # the FP32r verifier requires the full producer chain of f32r matmul
    # operands to be f32r-typed; dt.np(float32r) is np.float32, so the
    # host-side in_maps still pass plain fp32 arrays.
    a = nc.dram_tensor("a", [m_loc, K], tr_dt, kind="ExternalInput").ap()
    w = nc.dram_tensor("w", [N, K], tr_dt, kind="ExternalInput").ap()
    out = nc.dram_tensor("out", [m_loc, N], F32, kind="ExternalOutput").ap()

    # Block layout: element (p, r, k) = a[p*r_total + r, k] — partition p
    # owns r_total consecutive DRAM rows, so any r-slice ("chunk") is
    # contiguous HBM per partition and chunk sizes are free to vary.
    a_v = a.rearrange("(p r) k -> p r k", p=P)
    out_v = out.rearrange("(p r) n -> p r n", p=P)

    with tile.TileContext(nc) as tc:
        with (
            tc.tile_pool(name="const", bufs=1) as const_pool,
            tc.tile_pool(name="a_nat", bufs=a_bufs) as a_pool,
            tc.tile_pool(name="at", bufs=max(4, lag + 2)) as at_pool,
            tc.tile_pool(name="out_sb", bufs=out_bufs) as out_pool,
            tc.tile_pool(name="psum_t", bufs=4, space="PSUM") as psum_t_pool,
            tc.tile_pool(name="psum_mm", bufs=4, space="PSUM") as psum_mm_pool,
        ):
            # the FP32r BIR verifier requires every producer of an f32r
            # matmul operand to emit f32r; gpsimd memset/affine_select can't,
            # so build the identity in f32 and round-cast it once on DVE
            # (0.0/1.0 are exact in any fp format).
            if tr_dt == F32:
                identity = const_pool.tile([P, P], F32)
                make_identity(nc, identity)
            else:
                identity_f32 = const_pool.tile([P, P], F32)
                make_identity(nc, identity_f32)
                identity = const_pool.tile([P, P], tr_dt)
                nc.vector.tensor_copy(out=identity, in_=identity_f32)

            # --- first A chunk load goes ahead of the weight load on the SP
            # ring: the A stream is the long pole, so its first descriptors
            # should hit the engines first.
            chunks = _chunk_schedule(r_total, rp)
            a_tiles = [None] * len(chunks)
            chunk_base = []
            b = 0
            for rc in chunks:
                chunk_base.append(b)
                b += rc
            a_tiles[0] = a_pool.tile([P, chunks[0], K], tr_dt, tag="a_nat", name="a_nat")
            nc.sync.dma_start(out=a_tiles[0], in_=a_v[:, 0 : chunks[0], :])

            # --- one-time: wt[k partitions, kt, n] = w[n, kt*128 + k] ---
            w_nat = const_pool.tile([P, NT, K], tr_dt)
            nc.sync.dma_start(out=w_nat, in_=w.rearrange("(nt p) k -> p nt k", p=P))
            wt_sb = const_pool.tile([P, KT, N], mm_dt)
            for kt in range(KT):
                ps = psum_t_pool.tile([P, N], tr_dt, tag="ps_t")
                for nt in range(NT):
                    nc.tensor.transpose(
                        ps[:, nt * P : (nt + 1) * P],
                        w_nat[:, nt, kt * P : (kt + 1) * P],
                        identity,
                    )
                nc.vector.tensor_copy(out=wt_sb[:, kt, :], in_=ps)

            # --- main loop, software-pipelined ---
            # Front half of a stripe-pair iteration: 4 PE transposes into one
            # PSUM bank, then one DVE round-cast eviction to the f32r at-tile.
            # Back half (emitted `lag` iterations later): 4 accumulating
            # matmuls and the ACT eviction of the [m, n] result.
            #
            # Store triggers are DELAYED by `store_delay` chunks: the output
            # accumulates in the big out_sb ring, so when the load stream
            # runs out near the end, the store queue holds several MB of
            # ready-to-drain work and the SDMA engines stay at full rate
            # instead of dribbling at compute pace.
            pending = []
            store_q = []  # (chunk_idx, store_ap, out_sb_tile)

            def emit_store():
                _, dst, src = store_q.pop(0)
                if store_scalar:
                    nc.scalar.dma_start(out=dst, in_=src)
                else:
                    nc.gpsimd.dma_start(out=dst, in_=src)

            def emit_back_half():
                d = pending.pop(0)
                ps_mm = psum_mm_pool.tile([P, 2, N], F32, tag="ps_mm")
                for dr in range(2):
                    for kt in range(KT):
                        nc.tensor.matmul(
                            ps_mm[:, dr, :],
                            d["at"][:, dr, kt, :],
                            wt_sb[:, kt, :],
                            start=(kt == 0),
                            stop=(kt == KT - 1),
                        )
                nc.scalar.copy(out=d["dst"], in_=ps_mm)
                if d["store"] is not None:
                    store_q.append((d["ci"], d["store"], d["out_sb"]))
                while store_q and store_q[0][0] <= d["ci"] - store_delay:
                    emit_store()

            for ci, rc in enumerate(chunks):
                r_base = chunk_base[ci]
                if a_tiles[ci] is None:
                    a_tiles[ci] = a_pool.tile([P, rc, K], tr_dt, tag="a_nat", name="a_nat")
                    nc.sync.dma_start(
                        out=a_tiles[ci], in_=a_v[:, r_base : r_base + rc, :]
                    )
                a_nat = a_tiles[ci]
                out_sb = out_pool.tile([P, rc, N], F32, tag="out_sb")
                for r0 in range(0, rc, 2):
                    ps_t = psum_t_pool.tile([P, 2, KT, P], tr_dt, tag="ps_t")
                    for dr in range(2):
                        for kt in range(KT):
                            nc.tensor.transpose(
                                ps_t[:, dr, kt, :],
                                a_nat[:, r0 + dr, kt * P : (kt + 1) * P],
                                identity,
                            )
                    at = at_pool.tile([P, 2, KT, P], mm_dt, tag="at")
                    nc.vector.tensor_copy(out=at, in_=ps_t)
                    last = r0 + 2 >= rc
                    pending.append(
                        {
                            "at": at,
                            "dst": out_sb[:, r0 : r0 + 2, :],
                            "store": out_v[:, r_base : r_base + rc, :] if last else None,
                            "out_sb": out_sb,
                            "ci": ci,
                        }
                    )
                    if len(pending) > lag:
                        emit_back_half()
            while pending:
                emit_back_half()
            while store_q:
                emit_store()

    nc.compile()
    return nc


_NC_CACHE = {}


def _get_nc(**kw):
    key = tuple(sorted(kw.items()))
    if key not in _NC_CACHE:
        _NC_CACHE[key] = build_nc(**kw)
    return _NC_CACHE[key]


def run(inputs, trace=False, **build_kw):
    """Shard, run on 8 cores, gather. Returns (output, BassKernelResults)."""
    inp = np.ascontiguousarray(np.asarray(inputs["input"], dtype=np.float32))
    w = np.ascontiguousarray(np.asarray(inputs["weight"], dtype=np.float32))
    assert inp.shape == (M, K) and w.shape == (N, K)

    nc = _get_nc(**build_kw)
    shards = np.split(inp, NCORES, axis=0)
    in_maps = [{"a": shards[i], "w": w} for i in range(NCORES)]
    res = run_bass_kernel_spmd(nc, in_maps, list(range(NCORES)), trace=trace)
    out = np.concatenate([res.results[i]["out"] for i in range(NCORES)], axis=0)
    return out, res


def kernel(**inputs) -> np.ndarray:
    out, _ = run(inputs)
    return out
